# revision 11
# baseline (speedup 1.0000x reference)
"""GNN message passing (copy_src + segment_sum + Linear + ReLU) on 8 TRN2 cores.

v3: paired-row gather + identity one-hot tiles + batch-decoupled update.

Structure: dst nodes are packed (host side) into 392 windows = 8 cores x 49
slots, <=128 nodes per window. Per window there are 7 gather tiles of 128
descriptors; each 512B descriptor (elem_size=256 bf16, elem_step=128 ->
table rows j, j+1) carries TWO edges' src rows, dodging the sub-512B SDMA
read-modify-write penalty (halves DMA-engine and Q7 descriptor-gen time vs
per-edge 256B gathers). A gather tile [128, 256] is two virtual edge tiles
(A = cols 0:128, B = 128:256).

Scatter within a window:
- 5 IDENTITY tile-pairs (vtiles 0..9): vtile v holds the v-th edge of the
  node on each lane, so slot p scatters to lane p and the matmul rhs is a
  constant identity matrix — no one-hot needed. Same-node edge pairs share
  one descriptor (their src rows are laid out adjacently in the per-core
  HBM table); lanes with <2v edges point their descriptor at zero rows
  (contribute nothing). Covers min(deg,10) edges per node (~73%).
- 2 TAIL tiles: remaining edges (deg>10 spill) packed densely with
  arbitrary lanes; their one-hots are built on device by one DVE
  tensor_tensor is_equal per batch (iota vs dst-lane values, stride-0
  broadcast APs).

PE accumulates aggT[f, lane] += vtile[e, f].T @ rhs[e, lane] in PSUM
(fp32), 14 matmuls per window, with all 8 windows of a batch accumulated
before the update stage so PE's in-order queue stalls at most once per
batch (not once per window). Node update per batch: ACT copies each aggT
to a contiguous bf16 tile, then W^T is the loaded weight for two 512-wide
matmuls (out2T[fout, lane] = W @ aggT), and ACT applies bias+ReLU with a
per-partition bias column. Output is stored transposed [128, 6272] and
transposed back on the host during assembly.

Self-contained: shapes hardcoded for feature[50000,128], src/dst[640000],
W[128,128], b[128].
"""
import numpy as np
import ml_dtypes

import concourse.bacc as bacc
import concourse.tile as tile
from concourse import mybir
from concourse.bass_utils import run_bass_kernel_spmd

P = 128
N_NODES = 50000
N_EDGES = 640000
NC = 8
W_SLOTS = 49
NBINS = NC * W_SLOTS                 # 392 windows
CALL_TILES = 8                       # 1024 descriptors per dma_gather call
NQ = 4                               # SWDGE queues
BATCH_SLOTS = 8
C_ID = 5                             # identity tile-pairs per window
C_DVE = 2                            # tail tiles per window
C_TILES = C_ID + C_DVE               # 7 gather tiles per window
ID_EDGES = 2 * C_ID                  # identity edges per lane (vtiles 0..9)
TAIL_CAP = 2 * C_DVE * P             # tail edge capacity per window (512)
# table parts: slot ranges, so int16 part-relative row indices stay small
PART_SLOTS = [(0, 16), (16, 32), (32, 49)]

F32 = mybir.dt.float32
BF16 = mybir.dt.bfloat16
I16 = mybir.dt.int16
BF = ml_dtypes.bfloat16


def _pack_nodes(deg, db, tcap):
    """Assign all nodes to NBINS bins: <=128 nodes and <=tcap tail edges
    (sum of max(0, deg-ID_EDGES)) per bin. Greedy, high tail-load first."""
    order = np.argsort(-(db * 256 + deg))
    t_left = np.full(NBINS, tcap, dtype=np.int64)
    n_left = np.full(NBINS, P, dtype=np.int64)
    assign = np.empty(N_NODES, dtype=np.int64)
    for node in order:
        d = db[node]
        feas = (n_left > 0) & (t_left >= d)
        if not feas.any():
            return None
        score = t_left * (P / tcap) + 0.5 * n_left
        score[~feas] = -1e18
        bsel = int(np.argmax(score))
        assign[node] = bsel
        t_left[bsel] -= d
        n_left[bsel] -= 1
    return assign


def _make_plan(src, dst):
    src = np.asarray(src, dtype=np.int64)
    dst = np.asarray(dst, dtype=np.int64)
    deg = np.bincount(dst, minlength=N_NODES)
    db = np.maximum(deg - ID_EDGES, 0)

    for margin in (24, 12, 4):
        assign = _pack_nodes(deg, db, TAIL_CAP - margin)
        if assign is not None:
            break
    else:
        raise RuntimeError("node packing failed")

    bins = [np.where(assign == b)[0] for b in range(NBINS)]
    node_lane = np.empty(N_NODES, dtype=np.int64)
    for nodes in bins:
        node_lane[nodes] = np.arange(len(nodes))

    # edges sorted by (bin, lane) once
    ebin = assign[dst]
    order = np.lexsort((node_lane[dst], ebin))
    e_bin = ebin[order]
    e_src = src[order]
    e_lane = node_lane[dst[order]]
    starts = np.concatenate([[0], np.cumsum(np.bincount(e_bin,
                                                        minlength=NBINS))])

    T_tot = W_SLOTS * C_TILES
    part_of_slot = np.empty(W_SLOTS, dtype=np.int64)
    for pi, (s0, s1) in enumerate(PART_SLOTS):
        part_of_slot[s0:s1] = pi

    # tables[c][pi]: list of src node ids; -1 = zero row. Rows 0,1 reserved
    # as zeros (deadweight descriptors point at idx 0).
    tables = [[[-1, -1] for _ in range(len(PART_SLOTS))] for _ in range(NC)]
    idx_flat = np.zeros((NC, T_tot * P), dtype=np.int64)
    ndvec = 2 * C_DVE * W_SLOTS
    dstloc = np.full((NC, P, ndvec), -1.0, dtype=np.float32)

    for c in range(NC):
        for s in range(W_SLOTS):
            bid = c * W_SLOTS + s
            pi = part_of_slot[s]
            tab = tables[c][pi]
            e0, e1 = starts[bid], starts[bid + 1]
            srcs = e_src[e0:e1]
            lanes = e_lane[e0:e1]
            t_base = s * C_TILES
            lane_start = np.searchsorted(lanes, np.arange(P + 1))
            tail = []
            for lane in range(P):
                ls, le = int(lane_start[lane]), int(lane_start[lane + 1])
                d = le - ls
                nid = min(d, ID_EDGES)
                for v2 in range(0, nid, 2):
                    pos = len(tab)
                    tab.append(int(srcs[ls + v2]))
                    if v2 + 1 < nid:
                        tab.append(int(srcs[ls + v2 + 1]))
                    else:
                        tab.append(-1)
                    t = t_base + v2 // 2
                    idx_flat[c, t * P + lane] = pos
                for j in range(ls + nid, le):
                    tail.append((int(srcs[j]), int(lanes[j])))
            assert len(tail) <= TAIL_CAP, (c, s, len(tail))
            for j in range(0, len(tail), 2):
                pos = len(tab)
                tab.append(tail[j][0])
                if j + 1 < len(tail):
                    tab.append(tail[j + 1][0])
                else:
                    tab.append(-1)
                d2 = j // 2
                t = t_base + C_ID + d2 // P
                p = d2 % P
                idx_flat[c, t * P + p] = pos
                col = 2 * C_DVE * s + 2 * (d2 // P)
                dstloc[c, p, col] = tail[j][1]
                if j + 1 < len(tail):
                    dstloc[c, p, col + 1] = tail[j + 1][1]

    R = [max(len(tables[c][pi]) for c in range(NC)) + 2
         for pi in range(len(PART_SLOTS))]
    assert all(r < 32000 for r in R), R

    batches = []
    s = 0
    while s < W_SLOTS:
        s1 = min(s + BATCH_SLOTS, W_SLOTS)
        batches.append(dict(slots=list(range(s, s1)),
                            t_base=s * C_TILES,
                            T_b=(s1 - s) * C_TILES,
                            part=int(part_of_slot[s])))
        assert part_of_slot[s] == part_of_slot[s1 - 1]
        s = s1

    return dict(bins=bins, tables=tables, R=R, idx_flat=idx_flat,
                dstloc=dstloc, T_tot=T_tot, batches=batches)


def _wrap16(idx_flat):
    n = idx_flat.shape[0]
    arr = np.empty((16, n // 16), dtype=np.int16)
    j = np.arange(n)
    arr[j % 16, j // 16] = idx_flat
    return np.tile(arr, (8, 1))


def _overlap_ap(t, rows):
    """AP over a [rows+2, P] table reading 256 elems per row step of 128."""
    ap = t[:]
    v = ap.ap
    v[0] = (P, rows)
    v[1] = (1, 2 * P)
    ap.ap = v
    return ap


def _build_nc(plan, repeat=1):
    T_tot = plan["T_tot"]
    T_b0 = plan["batches"][0]["T_b"]
    ndvec = 2 * C_DVE * W_SLOTS
    # consts fp32 column layout:
    # [idx_b0 | idx_rest | dstloc | iota | ident | W^T | bias_col]
    c_i0 = 0                                  # idx batch0 int16 [P, T_b0*8]
    c_ir = c_i0 + T_b0 * 4                    # idx rest
    c_dl = c_ir + (T_tot - T_b0) * 4          # dstloc f32 [P, ndvec]
    c_io = c_dl + ndvec                       # iota f32 [P, P]
    c_id = c_io + P                           # identity bf16 [P, P]
    c_wt = c_id + P // 2                      # W^T bf16 [P, P]
    c_bc = c_wt + P // 2                      # bias col f32 [P, 1]
    c_tot = c_bc + 1
    plan["c_layout"] = (c_i0, c_ir, c_dl, c_io, c_id, c_wt, c_bc, c_tot)

    nc = bacc.Bacc("TRN2", num_swdge_queues=NQ)
    featP = [nc.declare_dram_parameter(f"featP{k}", [plan["R"][k] + 2, P],
                                       BF16, isOutput=False)
             for k in range(len(PART_SLOTS))]
    consts = nc.declare_dram_parameter("consts", [P, c_tot], F32,
                                       isOutput=False)
    out = nc.declare_dram_parameter("out", [P, W_SLOTS * P], F32,
                                    isOutput=True)
    feat_aps = [_overlap_ap(featP[k], plan["R"][k]) for k in
                range(len(PART_SLOTS))]

    with tile.TileContext(nc) as tc:
        with (
            tc.tile_pool(name="const", bufs=1) as const_pool,
            tc.tile_pool(name="msgs", bufs=3) as msgs_pool,
            tc.tile_pool(name="oneh", bufs=3) as oneh_pool,
            tc.tile_pool(name="outp", bufs=3) as out_pool,
            tc.tile_pool(name="psA", bufs=4, space="PSUM") as psum_agg,
            tc.tile_pool(name="psO", bufs=2, space="PSUM") as psum_out,
        ):
            # batch 0's indices land first so gathers start immediately
            cs0 = const_pool.tile([P, c_ir - c_i0], F32, tag="cs_b0")
            nc.sync.dma_start(out=cs0[:], in_=consts[:, c_i0:c_ir])
            cs = const_pool.tile([P, c_id - c_ir], F32, tag="cs_rest")
            nc.sync.dma_start(out=cs[:], in_=consts[:, c_ir:c_id])
            csm = const_pool.tile([P, c_tot - c_id], F32, tag="cs_misc")
            nc.sync.dma_start(out=csm[:], in_=consts[:, c_id:c_tot])
            idx0_sb = cs0[:].bitcast(I16)
            idxr_sb = cs[:, 0:c_dl - c_ir].bitcast(I16)
            dstloc_sb = cs[:, c_dl - c_ir:c_io - c_ir]
            iota_sb = cs[:, c_io - c_ir:c_id - c_ir]
            ident_sb = csm[:, 0:c_wt - c_id].bitcast(BF16)
            wt_sb = csm[:, c_wt - c_id:c_bc - c_id].bitcast(BF16)
            bcol_sb = csm[:, c_bc - c_id:c_tot - c_id]

            gq = [0]
            _rep_batches = [bt for _ in range(repeat)
                            for bt in plan["batches"]]

            for bt in _rep_batches:
                T_b = bt["T_b"]
                t0 = bt["t_base"]
                slots = bt["slots"]
                nwin = len(slots)
                w0 = slots[0]
                fap = feat_aps[bt["part"]]
                msgs = msgs_pool.tile([P, T_b, 2 * P], BF16, tag="msgs")
                for off in range(0, T_b, CALL_TILES):
                    nk = min(CALL_TILES, T_b - off)
                    if t0 == 0:
                        ia = idx0_sb[:, (t0 + off) * 8:(t0 + off + nk) * 8]
                    else:
                        ia = idxr_sb[:, (t0 - T_b0 + off) * 8:
                                     (t0 - T_b0 + off + nk) * 8]
                    nc.gpsimd.dma_gather(
                        out_ap=msgs[:, off:off + nk, :],
                        in_ap=fap,
                        idxs_ap=ia,
                        num_idxs=nk * P,
                        num_idxs_reg=nk * P,
                        elem_size=2 * P,
                        elem_step=P,
                        queue_num=gq[0] % NQ,
                    )
                    gq[0] += 1

                ncol = nwin * 2 * C_DVE
                oh = oneh_pool.tile([P, ncol, P], BF16, tag="onehot")
                nc.vector.tensor_tensor(
                    out=oh[:],
                    in0=iota_sb.unsqueeze(1).broadcast_to([P, ncol, P]),
                    in1=dstloc_sb[:, 2 * C_DVE * w0:2 * C_DVE * w0 + ncol]
                        .unsqueeze(2).broadcast_to([P, ncol, P]),
                    op=mybir.AluOpType.is_equal,
                )

                aggT_all = out_pool.tile([P, nwin, P], BF16, tag="aggT_all")
                out_sb = out_pool.tile([P, nwin, P], F32, tag="out_sb")
                for h in range(0, nwin, 4):
                    h1 = min(h + 4, nwin)
                    for wi in range(h, h1):
                        aggT_ps = psum_agg.tile([P, P], F32, tag="aggT")
                        tl = wi * C_TILES
                        for i in range(C_ID):
                            t = tl + i
                            nc.tensor.matmul(out=aggT_ps[:],
                                             lhsT=msgs[:, t, 0:P],
                                             rhs=ident_sb,
                                             start=(i == 0), stop=False)
                            nc.tensor.matmul(out=aggT_ps[:],
                                             lhsT=msgs[:, t, P:2 * P],
                                             rhs=ident_sb,
                                             start=False, stop=False)
                        for j in range(C_DVE):
                            t = tl + C_ID + j
                            oc = wi * 2 * C_DVE + 2 * j
                            nc.tensor.matmul(out=aggT_ps[:],
                                             lhsT=msgs[:, t, 0:P],
                                             rhs=oh[:, oc, :],
                                             start=False, stop=False)
                            nc.tensor.matmul(out=aggT_ps[:],
                                             lhsT=msgs[:, t, P:2 * P],
                                             rhs=oh[:, oc + 1, :],
                                             start=False,
                                             stop=(j == C_DVE - 1))
                        nc.scalar.activation(
                            out=aggT_all[:, wi, :], in_=aggT_ps[:],
                            func=mybir.ActivationFunctionType.Copy)

                    out2_ps = psum_out.tile([P, (h1 - h) * P], F32,
                                            tag="out2")
                    nc.tensor.matmul(
                        out=out2_ps[:],
                        lhsT=wt_sb,
                        rhs=aggT_all[:, h:h1, :]
                            .rearrange("p a b -> p (a b)"),
                        start=True, stop=True)
                    nc.scalar.activation(
                        out=out_sb[:, h:h1, :]
                            .rearrange("p a b -> p (a b)"),
                        in_=out2_ps[:],
                        func=mybir.ActivationFunctionType.Relu,
                        bias=bcol_sb[:, 0:1])
                nc.scalar.dma_start(
                    out=out[:, w0 * P:(w0 + nwin) * P],
                    in_=out_sb[:].rearrange("p a b -> p (a b)"))
    nc.finalize()
    return nc


_CACHE = {}


def _prepare(feature, src, dst, W, b):
    feature = np.asarray(feature, dtype=np.float32)
    W = np.asarray(W, dtype=np.float32)
    b = np.asarray(b, dtype=np.float32)
    key = (hash(np.asarray(src).tobytes()), hash(np.asarray(dst).tobytes()))
    if key not in _CACHE:
        plan = _make_plan(src, dst)
        nc = _build_nc(plan)
        _CACHE.clear()
        _CACHE[key] = (plan, nc)
    plan, nc = _CACHE[key]
    c_i0, c_ir, c_dl, c_io, c_id, c_wt, c_bc, c_tot = plan["c_layout"]
    T_tot = plan["T_tot"]
    T_b0 = plan["batches"][0]["T_b"]
    featbf = feature.astype(BF)

    def put_bf16(consts, col0, arr2d):
        a = np.asarray(arr2d, dtype=BF)
        rows, cols = a.shape
        pad = (-cols) % 2
        if pad:
            a = np.concatenate([a, np.zeros((rows, pad), BF)], axis=1)
        a = np.ascontiguousarray(a)
        consts[:rows, col0:col0 + a.shape[1] // 2] = a.view(np.float32)

    in_maps = []
    for c in range(NC):
        consts = np.zeros((P, c_tot), dtype=np.float32)
        wi = _wrap16(plan["idx_flat"][c].astype(np.int16)).view(np.float32)
        consts[:, c_i0:c_ir] = wi[:, :T_b0 * 4]
        consts[:, c_ir:c_dl] = wi[:, T_b0 * 4:]
        consts[:, c_dl:c_io] = plan["dstloc"][c]
        consts[:, c_io:c_id] = np.tile(np.arange(P, dtype=np.float32), (P, 1))
        put_bf16(consts, c_id, np.eye(P, dtype=np.float32))
        put_bf16(consts, c_wt, W.T.astype(BF))
        consts[:, c_bc] = b
        im = {"consts": consts}
        for k in range(len(PART_SLOTS)):
            tab = np.zeros((plan["R"][k] + 2, P), dtype=BF)
            rows = np.asarray(plan["tables"][c][k], dtype=np.int64)
            if len(rows):
                real = rows >= 0
                tab[:len(rows)][real] = featbf[rows[real]]
            im[f"featP{k}"] = tab
        in_maps.append(im)
    return plan, nc, in_maps


def _assemble(plan, results):
    out_full = np.zeros((N_NODES, P), dtype=np.float32)
    for c in range(NC):
        oc = results[c]["out"]
        for s in range(W_SLOTS):
            nodes = plan["bins"][c * W_SLOTS + s]
            if len(nodes):
                out_full[nodes] = oc[:, s * P:s * P + len(nodes)].T
    return out_full


def kernel(feature, src, dst, W, b):
    plan, nc, in_maps = _prepare(feature, src, dst, W, b)
    res = run_bass_kernel_spmd(nc, in_maps, list(range(NC)))
    return _assemble(plan, res.results)
